# revision 1
# baseline (speedup 1.0000x reference)
"""GroupQueryAttention TRN2 Bass kernel.

Problem: B=4, T=2048, C=1024, H=16 heads, G=4 groups, head_dim=64, causal.
Sharding: 8 cores = 4 batches (DP) x 2 tensor-parallel halves (8 heads /
2 groups each). Host pre-transposes x and weight slices; each core computes
a partial output projection over its 512 attention channels; host sums the
two TP partials per batch and adds the bias.

Device algorithm (per core; projections in fp32r, attention in bf16 --
mixed precision keeps rel err ~1.6e-3 while avoiding the fp32r power
throttle on the PE for the attention matmuls):
  qT[h] = WqT_h.T @ xT   (pair-packed: 2 heads per 128-partition tile)
  kT[g] likewise, duplicated onto both partition halves; vT transposed
  back to [T, 64] via PE into lhsT tiles [ones64 | v] so the PV matmul
  emits 64 replicated softmax-denominator rows (rows 0:64 of its psum)
  at zero extra PE cost.
  scoresT[tk, tq] = kT-block.T @ qT-block  (causal: skip/clip blocks;
  single-psum-bank tiles, bufs=4, so the PE streams ahead across heads)
  pT = exp(scoresT * 0.125) bf16 (ACT, PSUM->SBUF; no max-subtraction --
  scores are O(1)); diagonal 128-blocks masked by an upper-tri 0/1 mask.
  po[0:64] = replicated denom, po[64:128] = outT (PV accumulation).
  normalize: rcp = reciprocal_approx_fast(po[0:64]) (base-partition 0!);
  attnT = po[64:128] * rcp;  y[tq, :] += attnT.T @ WpT (partial, fp32).
Host sums the two TP partials per batch and adds the bias.
"""

import sys
import numpy as np
import ml_dtypes

for _p in ("/opt/trn_rl_repo", "/opt/trn_rl_repo/concourse"):
    if _p not in sys.path:
        sys.path.insert(0, _p)

import concourse.bass as bass  # noqa: E402
import concourse.mybir as mybir  # noqa: E402
from concourse import bacc  # noqa: E402
from concourse.tile import TileContext  # noqa: E402
from concourse.bass_utils import run_bass_kernel_spmd  # noqa: E402
from concourse.masks import make_identity, make_upper_triangular  # noqa: E402

F32 = mybir.dt.float32
F32R = mybir.dt.float32r
BF16 = mybir.dt.bfloat16

B, T, C = 4, 2048, 1024
NH, NG, HD = 16, 4, 64
NH_LOC, NG_LOC = 8, 2          # per-core heads / groups
S = NH_LOC * HD                # 512 local attention channels
TQB = 512                      # tq block
NTQB = T // TQB                # 4
NKT = T // 128                 # 16 tk tiles
NCT = C // 128                 # 8 contraction tiles
SCALE = float(HD) ** -0.5


def _build_program(trace_scopes=False):
    nc = bacc.Bacc("TRN2", target_bir_lowering=False, debug=False, num_devices=8)

    xT = nc.dram_tensor("xT", [C, T], F32R, kind="ExternalInput")
    wqT = nc.dram_tensor("wqT", [C, S], F32R, kind="ExternalInput")
    wkT = nc.dram_tensor("wkT", [C, NG_LOC * HD], F32R, kind="ExternalInput")
    wvT = nc.dram_tensor("wvT", [C, NG_LOC * HD], F32R, kind="ExternalInput")
    wpT = nc.dram_tensor("wpT", [S, C], F32R, kind="ExternalInput")
    y = nc.dram_tensor("y", [T, C], F32, kind="ExternalOutput")

    with TileContext(nc) as tc:
        with tc.tile_pool(name="const", bufs=1) as const_pool, \
             tc.tile_pool(name="persist", bufs=1) as persist, \
             tc.tile_pool(name="dram", bufs=4, space="DRAM") as dram_pool:

            ident = const_pool.tile([128, 64], F32)
            make_identity(nc, ident[0:64, 0:64])
            make_identity(nc, ident[64:128, 0:64], nomemset=False)
            mask32 = const_pool.tile([128, 128], F32)
            make_upper_triangular(nc, mask32, val=1.0, diag=True)
            mask = const_pool.tile([128, 128], BF16)
            nc.vector.tensor_copy(mask, mask32)
            ones64 = const_pool.tile([128, 64], F32)
            nc.vector.memset(ones64, 1.0)

            # ---- persistent SBUF tensors ----
            qt_sb = [persist.tile([128, T], BF16, tag=f"qt{i}", name=f"qt{i}") for i in range(4)]
            kdup = [persist.tile([128, T], BF16, tag=f"kd{g}", name=f"kd{g}")
                    for g in range(NG_LOC)]
            # v (transposed back): per group 16 tiles [128, 128]; first 64
            # lhsT cols are ones so PV emits 64 replicated denominator rows
            v_sb = [persist.tile([128, NKT * 128], BF16, tag=f"v{g}", name=f"v{g}")
                    for g in range(NG_LOC)]
            wp_sb = [persist.tile([128, C], F32R, tag=f"wp{i}", name=f"wp{i}") for i in range(4)]
            for i in range(4):
                nc.sync.dma_start(out=wp_sb[i], in_=wpT[i * 128:(i + 1) * 128, :])
            for g in range(NG_LOC):
                for t in range(NKT):
                    nc.vector.tensor_copy(
                        v_sb[g][:, t * 128:t * 128 + 64], ones64)

            # ================= Phase A: projections =================
            with tc.tile_pool(name="xw", bufs=1) as xw, \
                 tc.tile_pool(name="psA", bufs=4, space="PSUM") as psA:
                xts = [xw.tile([128, T], F32R, tag=f"x{ct}", name=f"x{ct}") for ct in range(NCT)]
                wq_sb = [xw.tile([128, S], F32R, tag=f"wq{ct}", name=f"wq{ct}") for ct in range(NCT)]
                wk_sb = [xw.tile([128, NG_LOC * HD], F32R, tag=f"wk{ct}", name=f"wk{ct}")
                         for ct in range(NCT)]
                wv_sb = [xw.tile([128, NG_LOC * HD], F32R, tag=f"wv{ct}", name=f"wv{ct}")
                         for ct in range(NCT)]
                for ct in range(NCT):
                    rows = slice(ct * 128, (ct + 1) * 128)
                    nc.sync.dma_start(out=xts[ct], in_=xT[rows, :])
                    nc.sync.dma_start(out=wq_sb[ct], in_=wqT[rows, :])
                    nc.sync.dma_start(out=wk_sb[ct], in_=wkT[rows, :])
                    nc.sync.dma_start(out=wv_sb[ct], in_=wvT[rows, :])

                # qT: 4 head-pairs x 4 tq blocks, accumulate over 8 c-tiles
                for p4 in range(4):
                    for j in range(NTQB):
                        ps = psA.tile([128, TQB], F32, tag="psA")
                        for ct in range(NCT):
                            nc.tensor.matmul(
                                ps,
                                wq_sb[ct][:, p4 * 128:(p4 + 1) * 128],
                                xts[ct][:, j * TQB:(j + 1) * TQB],
                                start=(ct == 0), stop=(ct == NCT - 1))
                        nc.scalar.copy(qt_sb[p4][:, j * TQB:(j + 1) * TQB], ps)

                # kT: one pair (2 groups); duplicate each group onto both
                # partition halves (matmul operands must share base_partition)
                for j in range(NTQB):
                    ps = psA.tile([128, TQB], F32, tag="psA")
                    for ct in range(NCT):
                        nc.tensor.matmul(
                            ps, wk_sb[ct], xts[ct][:, j * TQB:(j + 1) * TQB],
                            start=(ct == 0), stop=(ct == NCT - 1))
                    cols = slice(j * TQB, (j + 1) * TQB)
                    nc.scalar.copy(kdup[0][0:64, cols], ps[0:64, :])
                    nc.scalar.copy(kdup[1][64:128, cols], ps[64:128, :])
                nc.sync.dma_start(out=kdup[0][64:128, :], in_=kdup[0][0:64, :])
                nc.sync.dma_start(out=kdup[1][0:64, :], in_=kdup[1][64:128, :])

                # vT then PE-transpose into v_sb ([T,64] layout + ones col)
                vt_sb = xw.tile([128, T], F32, tag="vt")
                for j in range(NTQB):
                    ps = psA.tile([128, TQB], F32, tag="psA")
                    for ct in range(NCT):
                        nc.tensor.matmul(
                            ps, wv_sb[ct], xts[ct][:, j * TQB:(j + 1) * TQB],
                            start=(ct == 0), stop=(ct == NCT - 1))
                    nc.vector.tensor_copy(vt_sb[:, j * TQB:(j + 1) * TQB], ps)
                for g in range(NG_LOC):
                    for t in range(NKT):
                        pst = psA.tile([128, TQB], F32, tag="psA")
                        nc.tensor.transpose(
                            pst[:, 0:64],
                            vt_sb[g * 64:(g + 1) * 64, t * 128:(t + 1) * 128],
                            ident[g * 64:(g + 1) * 64, 0:64])
                        nc.vector.tensor_copy(
                            v_sb[g][:, t * 128 + 64:t * 128 + 128], pst[:, 0:64])

            # ================= Phase B: attention + proj =================
            with tc.tile_pool(name="pp", bufs=8) as ppool, \
                 tc.tile_pool(name="attn", bufs=8) as apool, \
                 tc.tile_pool(name="sm", bufs=4) as small, \
                 tc.tile_pool(name="yo", bufs=4) as ypool, \
                 tc.tile_pool(name="psS", bufs=4, space="PSUM") as psS, \
                 tc.tile_pool(name="psO", bufs=2, space="PSUM") as psO, \
                 tc.tile_pool(name="psP", bufs=2, space="PSUM") as psP:

                for j in range(NTQB):
                    tq0 = j * TQB
                    ntk = 4 * (j + 1)
                    at_j = [apool.tile([128, TQB], F32R, tag=f"at{p4}", name=f"at{p4}")
                            for p4 in range(4)]
                    for h in range(NH_LOC):
                        g = h // 4
                        p4, r = h // 2, h % 2
                        qT_h = qt_sb[p4][r * 64:(r + 1) * 64, :]
                        kT_g = kdup[g][r * 64:(r + 1) * 64, :]
                        po = psO.tile([128, TQB], F32, tag="po")
                        for t in range(ntk):
                            c = t - 4 * j
                            off = max(0, c * 128)
                            pscore = psS.tile([128, TQB], F32, tag="ps")
                            nc.tensor.matmul(
                                pscore[:, off:TQB],
                                kT_g[:, t * 128:(t + 1) * 128],
                                qT_h[:, tq0 + off:tq0 + TQB],
                                start=True, stop=True)
                            pt = ppool.tile([128, TQB], BF16, tag="pt")
                            nc.scalar.activation(
                                pt[:, off:TQB], pscore[:, off:TQB],
                                mybir.ActivationFunctionType.Exp, scale=SCALE)
                            if c >= 0:
                                nc.vector.tensor_mul(
                                    pt[:, off:off + 128],
                                    pt[:, off:off + 128], mask)
                            nc.tensor.matmul(
                                po[:, off:TQB],
                                v_sb[g][:, t * 128:(t + 1) * 128],
                                pt[:, off:TQB],
                                start=(t == 0), stop=(t == ntk - 1))
                        # normalization: recip -> DRAM -> broadcast -> mul
                        rcp = small.tile([128, TQB], F32, tag="recip")
                        nc.vector.reciprocal_approx_fast(rcp[0:64, :], po[0:64, :])
                        nc.vector.tensor_mul(
                            at_j[p4][r * 64:(r + 1) * 64, :],
                            po[64:128, :], rcp[0:64, :])
                    # output projection for this tq block
                    for tt in range(4):
                        tau = j * 4 + tt
                        ysb = ypool.tile([128, C], F32, tag="y")
                        for half in range(2):
                            yp = psP.tile([128, TQB], F32, tag="yp")
                            for p4 in range(4):
                                nc.tensor.matmul(
                                    yp,
                                    at_j[p4][:, tt * 128:(tt + 1) * 128],
                                    wp_sb[p4][:, half * TQB:(half + 1) * TQB],
                                    start=(p4 == 0), stop=(p4 == 3))
                            nc.vector.tensor_copy(
                                ysb[:, half * TQB:(half + 1) * TQB], yp)
                        nc.sync.dma_start(
                            out=y[tau * 128:(tau + 1) * 128, :], in_=ysb)

    nc.compile()
    return nc


_NC_CACHE = None


def _get_nc():
    global _NC_CACHE
    if _NC_CACHE is None:
        _NC_CACHE = _build_program()
    return _NC_CACHE


def _make_in_maps(x, Wq, Wk, Wv, Wp):
    in_maps = []
    for core in range(8):
        b, tp = core // 2, core % 2
        hs = slice(tp * NH_LOC, (tp + 1) * NH_LOC)
        gs = slice(tp * NG_LOC, (tp + 1) * NG_LOC)
        in_maps.append({
            "xT": np.ascontiguousarray(x[b].T),
            "wqT": np.ascontiguousarray(
                Wq[hs].transpose(2, 0, 1).reshape(C, S)),
            "wkT": np.ascontiguousarray(
                Wk[gs].transpose(2, 0, 1).reshape(C, NG_LOC * HD)),
            "wvT": np.ascontiguousarray(
                Wv[gs].transpose(2, 0, 1).reshape(C, NG_LOC * HD)),
            "wpT": np.ascontiguousarray(Wp[:, tp * S:(tp + 1) * S].T),
        })
    return in_maps


def kernel(x, Wq, Wk, Wv, Wp, bp, _trace=False):
    x = np.asarray(x, dtype=np.float32)
    nc = _get_nc()
    in_maps = _make_in_maps(
        x, np.asarray(Wq, np.float32), np.asarray(Wk, np.float32),
        np.asarray(Wv, np.float32), np.asarray(Wp, np.float32))
    res = run_bass_kernel_spmd(nc, in_maps, list(range(8)), trace=_trace)
    out = np.empty((B, T, C), dtype=np.float32)
    bp32 = np.asarray(bp, np.float32)
    for b in range(B):
        out[b] = res.results[2 * b]["y"] + res.results[2 * b + 1]["y"] + bp32
    if _trace:
        return out, res
    return out



# revision 5
# speedup vs baseline: 1.2328x; 1.2328x over previous
"""GroupQueryAttention TRN2 Bass kernel (v2: all-bf16, ct-outer phase A,
paired activations).

Problem: B=4, T=2048, C=1024, H=16 heads, G=4 groups, head_dim=64, causal.
Sharding: 8 cores = 4 batches (DP) x 2 tensor-parallel halves (8 heads /
2 groups each). Host pre-transposes x and weight slices to bf16; each core
computes a partial output projection over its 512 attention channels; host
sums the two TP partials per batch and adds the bias.

v2 changes vs baseline (323-378us):
  - Everything bf16 on device (was fp32r projections): fp32-mode HIGH
    matmuls draw more power and trip the HAM throttle; bf16 streams at
    ~0.5ns/row. Host ships x/weights already in bf16 (halves input DMA).
  - Phase A is ct-outer: KV pass then two Q passes, each accumulating
    8 PSUM banks across the 8 contraction tiles, so the PE starts on the
    first x tile instead of waiting for the full 8MB x DMA.
  - Scores PSUM tiles are [128,1024] pairs (2 banks); one Exp ACTIVATE
    per pair halves the Scalar engine's 352-cycle/instruction overhead
    (ACT is the attention-phase co-bottleneck at ~1 elem/cycle/lane
    @1.2GHz). Clipped diagonal members leave stale PSUM in the dead
    columns; exp of garbage is finite-or-inf and never read (PV clips).
"""

import sys
import numpy as np
import ml_dtypes

for _p in ("/opt/trn_rl_repo", "/opt/trn_rl_repo/concourse"):
    if _p not in sys.path:
        sys.path.insert(0, _p)

import concourse.bass as bass  # noqa: E402
import concourse.mybir as mybir  # noqa: E402
from concourse import bacc  # noqa: E402
from concourse.tile import TileContext  # noqa: E402
from concourse.bass_utils import run_bass_kernel_spmd  # noqa: E402
from concourse.masks import make_identity, make_upper_triangular  # noqa: E402

F32 = mybir.dt.float32
BF16 = mybir.dt.bfloat16
BFNP = ml_dtypes.bfloat16

B, T, C = 4, 2048, 1024
NH, NG, HD = 16, 4, 64
NH_LOC, NG_LOC = 8, 2          # per-core heads / groups
S = NH_LOC * HD                # 512 local attention channels
TQB = 512                      # tq block
NTQB = T // TQB                # 4
NKT = T // 128                 # 16 tk tiles
NCT = C // 128                 # 8 contraction tiles
SCALE = float(HD) ** -0.5


def _build_program():
    nc = bacc.Bacc("TRN2", target_bir_lowering=False, debug=False, num_devices=8)

    xT = nc.dram_tensor("xT", [C, T], BF16, kind="ExternalInput")
    wqT = nc.dram_tensor("wqT", [C, S], BF16, kind="ExternalInput")
    wkT = nc.dram_tensor("wkT", [C, NG_LOC * HD], BF16, kind="ExternalInput")
    wvT = nc.dram_tensor("wvT", [C, NG_LOC * HD], BF16, kind="ExternalInput")
    wpT = nc.dram_tensor("wpT", [S, C], BF16, kind="ExternalInput")
    y = nc.dram_tensor("y", [T, C], F32, kind="ExternalOutput")

    with TileContext(nc) as tc:
        with tc.tile_pool(name="const", bufs=1) as const_pool, \
             tc.tile_pool(name="persist", bufs=1) as persist:

            ident = const_pool.tile([128, 64], F32)
            make_identity(nc, ident[0:64, 0:64])
            make_identity(nc, ident[64:128, 0:64], nomemset=False)
            mask32 = const_pool.tile([128, 128], F32)
            make_upper_triangular(nc, mask32, val=1.0, diag=True)
            mask = const_pool.tile([128, 128], BF16)
            nc.vector.tensor_copy(mask, mask32)
            ones64 = const_pool.tile([128, 64], F32)
            nc.vector.memset(ones64, 1.0)

            # ---- persistent SBUF tensors ----
            qt_sb = [persist.tile([128, T], BF16, tag=f"qt{i}", name=f"qt{i}")
                     for i in range(4)]
            kdup = [persist.tile([128, T], BF16, tag=f"kd{g}", name=f"kd{g}")
                    for g in range(NG_LOC)]
            # v (transposed back): per group 16 tiles [128, 128]; first 64
            # lhsT cols are ones so PV emits 64 replicated denominator rows
            v_sb = [persist.tile([128, NKT * 128], BF16, tag=f"v{g}", name=f"v{g}")
                    for g in range(NG_LOC)]
            wp_sb = [persist.tile([128, C], BF16, tag=f"wp{i}", name=f"wp{i}")
                     for i in range(4)]
            for g in range(NG_LOC):
                for t in range(NKT):
                    nc.vector.tensor_copy(
                        v_sb[g][:, t * 128:t * 128 + 64], ones64)

            # ================= Phase A: projections =================
            with tc.tile_pool(name="xw", bufs=1) as xw, \
                 tc.tile_pool(name="psA", bufs=1, space="PSUM") as psA:
                xts = [xw.tile([128, T], BF16, tag=f"x{ct}", name=f"x{ct}")
                       for ct in range(NCT)]
                wq_sb = [xw.tile([128, S], BF16, tag=f"wq{ct}", name=f"wq{ct}")
                         for ct in range(NCT)]
                wk_sb = [xw.tile([128, NG_LOC * HD], BF16, tag=f"wk{ct}", name=f"wk{ct}")
                        for ct in range(NCT)]
                wv_sb = [xw.tile([128, NG_LOC * HD], BF16, tag=f"wv{ct}", name=f"wv{ct}")
                        for ct in range(NCT)]
                # ct-interleaved DMA issue so tile ct is complete before
                # the ct-th accumulation step of the kv pass
                for ct in range(NCT):
                    rows = slice(ct * 128, (ct + 1) * 128)
                    nc.sync.dma_start(out=xts[ct], in_=xT[rows, :])
                    nc.sync.dma_start(out=wk_sb[ct], in_=wkT[rows, :])
                    nc.sync.dma_start(out=wv_sb[ct], in_=wvT[rows, :])
                    nc.sync.dma_start(out=wq_sb[ct], in_=wqT[rows, :])
                for i in range(4):
                    nc.sync.dma_start(out=wp_sb[i], in_=wpT[i * 128:(i + 1) * 128, :])

                # ---- KV pass: 8 psum banks (k j0..3, v j0..3), ct-outer
                kps = [psA.tile([128, TQB], F32, tag=f"b{j}", name=f"pk{j}") for j in range(NTQB)]
                vps = [psA.tile([128, TQB], F32, tag=f"b{4 + j}", name=f"pv{j}") for j in range(NTQB)]
                for ct in range(NCT):
                    for j in range(NTQB):
                        cols = slice(j * TQB, (j + 1) * TQB)
                        nc.tensor.matmul(
                            kps[j], wk_sb[ct], xts[ct][:, cols],
                            start=(ct == 0), stop=(ct == NCT - 1))
                        nc.tensor.matmul(
                            vps[j], wv_sb[ct], xts[ct][:, cols],
                            start=(ct == 0), stop=(ct == NCT - 1))
                # k: duplicate each group onto both partition halves
                for j in range(NTQB):
                    cols = slice(j * TQB, (j + 1) * TQB)
                    nc.scalar.copy(kdup[0][0:64, cols], kps[j][0:64, :])
                    nc.scalar.copy(kdup[1][64:128, cols], kps[j][64:128, :])
                vt_sb = xw.tile([128, T], F32, tag="vt")
                for j in range(NTQB):
                    cols = slice(j * TQB, (j + 1) * TQB)
                    nc.vector.tensor_copy(vt_sb[:, cols], vps[j])
                nc.sync.dma_start(out=kdup[0][64:128, :], in_=kdup[0][0:64, :])
                nc.sync.dma_start(out=kdup[1][0:64, :], in_=kdup[1][64:128, :])

                # ---- Q pass 1 (j=0,1) then pass 2 (j=2,3): ct-outer
                for half in range(2):
                    qps = [[psA.tile([128, TQB], F32, tag=f"b{p4 * 2 + jj}", name=f"pq{p4}{jj}")
                            for jj in range(2)] for p4 in range(4)]
                    for ct in range(NCT):
                        for p4 in range(4):
                            for jj in range(2):
                                j = half * 2 + jj
                                nc.tensor.matmul(
                                    qps[p4][jj],
                                    wq_sb[ct][:, p4 * 128:(p4 + 1) * 128],
                                    xts[ct][:, j * TQB:(j + 1) * TQB],
                                    start=(ct == 0), stop=(ct == NCT - 1))
                    for p4 in range(4):
                        for jj in range(2):
                            j = half * 2 + jj
                            nc.scalar.copy(
                                qt_sb[p4][:, j * TQB:(j + 1) * TQB], qps[p4][jj])

                # ---- vT -> PE-transpose into v_sb ([T,64] layout + ones)
                for g in range(NG_LOC):
                    for t in range(NKT):
                        pst = psA.tile([128, TQB], F32, tag=f"b{(g * NKT + t) % 8}", name="pstT")
                        nc.tensor.transpose(
                            pst[:, 0:64],
                            vt_sb[g * 64:(g + 1) * 64, t * 128:(t + 1) * 128],
                            ident[g * 64:(g + 1) * 64, 0:64])
                        nc.vector.tensor_copy(
                            v_sb[g][:, t * 128 + 64:t * 128 + 128], pst[:, 0:64])

            # ================= Phase B: attention + proj =================
            with tc.tile_pool(name="pp", bufs=6) as ppool, \
                 tc.tile_pool(name="attn", bufs=8) as apool, \
                 tc.tile_pool(name="sm", bufs=4) as small, \
                 tc.tile_pool(name="yo", bufs=4) as ypool, \
                 tc.tile_pool(name="psS", bufs=2, space="PSUM") as psS, \
                 tc.tile_pool(name="psO", bufs=2, space="PSUM") as psO, \
                 tc.tile_pool(name="psP", bufs=2, space="PSUM") as psP:

                for j in range(NTQB):
                    tq0 = j * TQB
                    ntk = 4 * (j + 1)
                    npr = ntk // 2
                    at_j = [apool.tile([128, TQB], BF16, tag=f"at{p4}", name=f"at{p4}")
                            for p4 in range(4)]
                    for h in range(NH_LOC):
                        g = h // 4
                        p4, r = h // 2, h % 2
                        qT_h = qt_sb[p4][r * 64:(r + 1) * 64, :]
                        kT_g = kdup[g][r * 64:(r + 1) * 64, :]
                        po = psO.tile([128, TQB], F32, tag="po", name="po")
                        for pr in range(npr):
                            psc = psS.tile([128, 2 * TQB], F32, tag="ps", name="psc")
                            pt = ppool.tile([128, 2 * TQB], BF16, tag="pt", name="ptp")
                            offs = []
                            for m in range(2):
                                t = 2 * pr + m
                                c = t - 4 * j
                                off = max(0, c * 128)
                                offs.append(off)
                                nc.tensor.matmul(
                                    psc[:, m * TQB + off:(m + 1) * TQB],
                                    kT_g[:, t * 128:(t + 1) * 128],
                                    qT_h[:, tq0 + off:tq0 + TQB],
                                    start=True, stop=True)
                            # one Exp over both members (incl. dead cols)
                            nc.scalar.activation(
                                pt, psc,
                                mybir.ActivationFunctionType.Exp, scale=SCALE)
                            for m in range(2):
                                t = 2 * pr + m
                                c = t - 4 * j
                                off = offs[m]
                                if c >= 0:
                                    nc.vector.tensor_mul(
                                        pt[:, m * TQB + off:m * TQB + off + 128],
                                        pt[:, m * TQB + off:m * TQB + off + 128],
                                        mask)
                                nc.tensor.matmul(
                                    po[:, off:TQB],
                                    v_sb[g][:, t * 128:(t + 1) * 128],
                                    pt[:, m * TQB + off:(m + 1) * TQB],
                                    start=(t == 0), stop=(t == ntk - 1))
                        # normalization
                        rcp = small.tile([128, TQB], F32, tag="recip", name="rcp")
                        nc.vector.reciprocal_approx_fast(rcp[0:64, :], po[0:64, :])
                        nc.vector.tensor_mul(
                            at_j[p4][r * 64:(r + 1) * 64, :],
                            po[64:128, :], rcp[0:64, :])
                    # output projection for this tq block
                    for tt in range(4):
                        tau = j * 4 + tt
                        ysb = ypool.tile([128, C], F32, tag="y", name="ysb")
                        for half in range(2):
                            yp = psP.tile([128, TQB], F32, tag="yp", name="yp")
                            for p4 in range(4):
                                nc.tensor.matmul(
                                    yp,
                                    at_j[p4][:, tt * 128:(tt + 1) * 128],
                                    wp_sb[p4][:, half * TQB:(half + 1) * TQB],
                                    start=(p4 == 0), stop=(p4 == 3))
                            nc.vector.tensor_copy(
                                ysb[:, half * TQB:(half + 1) * TQB], yp)
                        nc.sync.dma_start(
                            out=y[tau * 128:(tau + 1) * 128, :], in_=ysb)

    nc.compile()
    return nc


_NC_CACHE = None


def _get_nc():
    global _NC_CACHE
    if _NC_CACHE is None:
        _NC_CACHE = _build_program()
    return _NC_CACHE


def _make_in_maps(x, Wq, Wk, Wv, Wp):
    in_maps = []
    for core in range(8):
        b, tp = core // 2, core % 2
        hs = slice(tp * NH_LOC, (tp + 1) * NH_LOC)
        gs = slice(tp * NG_LOC, (tp + 1) * NG_LOC)
        in_maps.append({
            "xT": np.ascontiguousarray(x[b].T.astype(BFNP)),
            "wqT": np.ascontiguousarray(
                Wq[hs].transpose(2, 0, 1).reshape(C, S).astype(BFNP)),
            "wkT": np.ascontiguousarray(
                Wk[gs].transpose(2, 0, 1).reshape(C, NG_LOC * HD).astype(BFNP)),
            "wvT": np.ascontiguousarray(
                Wv[gs].transpose(2, 0, 1).reshape(C, NG_LOC * HD).astype(BFNP)),
            "wpT": np.ascontiguousarray(
                Wp[:, tp * S:(tp + 1) * S].T.astype(BFNP)),
        })
    return in_maps


def kernel(x, Wq, Wk, Wv, Wp, bp, _trace=False):
    x = np.asarray(x, dtype=np.float32)
    nc = _get_nc()
    in_maps = _make_in_maps(
        x, np.asarray(Wq, np.float32), np.asarray(Wk, np.float32),
        np.asarray(Wv, np.float32), np.asarray(Wp, np.float32))
    res = run_bass_kernel_spmd(nc, in_maps, list(range(8)), trace=_trace)
    out = np.empty((B, T, C), dtype=np.float32)
    bp32 = np.asarray(bp, np.float32)
    for b in range(B):
        out[b] = res.results[2 * b]["y"] + res.results[2 * b + 1]["y"] + bp32
    if _trace:
        return out, res
    return out


# revision 6
# speedup vs baseline: 1.3669x; 1.1087x over previous
"""GroupQueryAttention TRN2 Bass kernel (v2: all-bf16, ct-outer phase A,
paired activations).

Problem: B=4, T=2048, C=1024, H=16 heads, G=4 groups, head_dim=64, causal.
Sharding: 8 cores = 4 batches (DP) x 2 tensor-parallel halves (8 heads /
2 groups each). Host pre-transposes x and weight slices to bf16; each core
computes a partial output projection over its 512 attention channels; host
sums the two TP partials per batch and adds the bias.

v2 changes vs baseline (323-378us):
  - Everything bf16 on device (was fp32r projections): fp32-mode HIGH
    matmuls draw more power and trip the HAM throttle; bf16 streams at
    ~0.5ns/row. Host ships x/weights already in bf16 (halves input DMA).
  - Phase A is ct-outer: KV pass then two Q passes, each accumulating
    8 PSUM banks across the 8 contraction tiles, so the PE starts on the
    first x tile instead of waiting for the full 8MB x DMA.
  - Scores PSUM tiles are [128,1024] pairs (2 banks); one Exp ACTIVATE
    per pair halves the Scalar engine's 352-cycle/instruction overhead
    (ACT is the attention-phase co-bottleneck at ~1 elem/cycle/lane
    @1.2GHz). Clipped diagonal members leave stale PSUM in the dead
    columns; exp of garbage is finite-or-inf and never read (PV clips).
"""

import sys
import numpy as np
import ml_dtypes

for _p in ("/opt/trn_rl_repo", "/opt/trn_rl_repo/concourse"):
    if _p not in sys.path:
        sys.path.insert(0, _p)

import concourse.bass as bass  # noqa: E402
import concourse.mybir as mybir  # noqa: E402
from concourse import bacc  # noqa: E402
from concourse.tile import TileContext  # noqa: E402
from concourse.bass_utils import run_bass_kernel_spmd  # noqa: E402
from concourse.masks import make_identity, make_upper_triangular  # noqa: E402

F32 = mybir.dt.float32
BF16 = mybir.dt.bfloat16
BFNP = ml_dtypes.bfloat16

B, T, C = 4, 2048, 1024
NH, NG, HD = 16, 4, 64
NH_LOC, NG_LOC = 8, 2          # per-core heads / groups
S = NH_LOC * HD                # 512 local attention channels
TQB = 512                      # tq block
NTQB = T // TQB                # 4
NKT = T // 128                 # 16 tk tiles
NCT = C // 128                 # 8 contraction tiles
SCALE = float(HD) ** -0.5


def _build_program():
    nc = bacc.Bacc("TRN2", target_bir_lowering=False, debug=False, num_devices=8)

    xT = nc.dram_tensor("xT", [C, T], BF16, kind="ExternalInput")
    wqT = nc.dram_tensor("wqT", [C, S], BF16, kind="ExternalInput")
    wkT = nc.dram_tensor("wkT", [C, NG_LOC * HD], BF16, kind="ExternalInput")
    wvT = nc.dram_tensor("wvT", [C, NG_LOC * HD], BF16, kind="ExternalInput")
    wpT = nc.dram_tensor("wpT", [S, C], BF16, kind="ExternalInput")
    y = nc.dram_tensor("y", [T, C], F32, kind="ExternalOutput")

    with TileContext(nc) as tc:
        with tc.tile_pool(name="const", bufs=1) as const_pool, \
             tc.tile_pool(name="persist", bufs=1) as persist:

            ident = const_pool.tile([128, 64], F32)
            make_identity(nc, ident[0:64, 0:64])
            make_identity(nc, ident[64:128, 0:64], nomemset=False)
            mask32 = const_pool.tile([128, 128], F32)
            make_upper_triangular(nc, mask32, val=1.0, diag=True)
            mask = const_pool.tile([128, 128], BF16)
            nc.vector.tensor_copy(mask, mask32)
            ones64 = const_pool.tile([128, 64], F32)
            nc.vector.memset(ones64, 1.0)

            # ---- persistent SBUF tensors ----
            qt_sb = [persist.tile([128, T], BF16, tag=f"qt{i}", name=f"qt{i}")
                     for i in range(4)]
            kdup = [persist.tile([128, T], BF16, tag=f"kd{g}", name=f"kd{g}")
                    for g in range(NG_LOC)]
            # v (transposed back): per group 16 tiles [128, 128]; first 64
            # lhsT cols are ones so PV emits 64 replicated denominator rows
            v_sb = [persist.tile([128, NKT * 128], BF16, tag=f"v{g}", name=f"v{g}")
                    for g in range(NG_LOC)]
            wp_sb = [persist.tile([128, C], BF16, tag=f"wp{i}", name=f"wp{i}")
                     for i in range(4)]
            for g in range(NG_LOC):
                for t in range(NKT):
                    nc.vector.tensor_copy(
                        v_sb[g][:, t * 128:t * 128 + 64], ones64)

            # ================= Phase A: projections =================
            with tc.tile_pool(name="xw", bufs=1) as xw, \
                 tc.tile_pool(name="psA", bufs=1, space="PSUM") as psA:
                xts = [xw.tile([128, T], BF16, tag=f"x{ct}", name=f"x{ct}")
                       for ct in range(NCT)]
                wq_sb = [xw.tile([128, S], BF16, tag=f"wq{ct}", name=f"wq{ct}")
                         for ct in range(NCT)]
                wk_sb = [xw.tile([128, NG_LOC * HD], BF16, tag=f"wk{ct}", name=f"wk{ct}")
                        for ct in range(NCT)]
                wv_sb = [xw.tile([128, NG_LOC * HD], BF16, tag=f"wv{ct}", name=f"wv{ct}")
                        for ct in range(NCT)]
                # ct-interleaved DMA issue so tile ct is complete before
                # the ct-th accumulation step of the kv pass
                for ct in range(NCT):
                    rows = slice(ct * 128, (ct + 1) * 128)
                    nc.sync.dma_start(out=xts[ct], in_=xT[rows, :])
                    nc.sync.dma_start(out=wk_sb[ct], in_=wkT[rows, :])
                    nc.sync.dma_start(out=wv_sb[ct], in_=wvT[rows, :])
                    nc.sync.dma_start(out=wq_sb[ct], in_=wqT[rows, :])
                for i in range(4):
                    nc.sync.dma_start(out=wp_sb[i], in_=wpT[i * 128:(i + 1) * 128, :])

                # ---- KV pass: 8 psum banks (k j0..3, v j0..3), ct-outer
                kps = [psA.tile([128, TQB], F32, tag=f"b{j}", name=f"pk{j}") for j in range(NTQB)]
                vps = [psA.tile([128, TQB], F32, tag=f"b{4 + j}", name=f"pv{j}") for j in range(NTQB)]
                for ct in range(NCT):
                    for j in range(NTQB):
                        cols = slice(j * TQB, (j + 1) * TQB)
                        nc.tensor.matmul(
                            kps[j], wk_sb[ct], xts[ct][:, cols],
                            start=(ct == 0), stop=(ct == NCT - 1))
                        nc.tensor.matmul(
                            vps[j], wv_sb[ct], xts[ct][:, cols],
                            start=(ct == 0), stop=(ct == NCT - 1))
                # k: duplicate each group onto both partition halves
                for j in range(NTQB):
                    cols = slice(j * TQB, (j + 1) * TQB)
                    nc.scalar.copy(kdup[0][0:64, cols], kps[j][0:64, :])
                    nc.scalar.copy(kdup[1][64:128, cols], kps[j][64:128, :])
                vt_sb = xw.tile([128, T], F32, tag="vt")
                for j in range(NTQB):
                    cols = slice(j * TQB, (j + 1) * TQB)
                    nc.vector.tensor_copy(vt_sb[:, cols], vps[j])
                nc.sync.dma_start(out=kdup[0][64:128, :], in_=kdup[0][0:64, :])
                nc.sync.dma_start(out=kdup[1][0:64, :], in_=kdup[1][64:128, :])

                # ---- Q pass 1 (j=0,1) then pass 2 (j=2,3): ct-outer
                for half in range(2):
                    qps = [[psA.tile([128, TQB], F32, tag=f"b{p4 * 2 + jj}", name=f"pq{p4}{jj}")
                            for jj in range(2)] for p4 in range(4)]
                    for ct in range(NCT):
                        for p4 in range(4):
                            for jj in range(2):
                                j = half * 2 + jj
                                nc.tensor.matmul(
                                    qps[p4][jj],
                                    wq_sb[ct][:, p4 * 128:(p4 + 1) * 128],
                                    xts[ct][:, j * TQB:(j + 1) * TQB],
                                    start=(ct == 0), stop=(ct == NCT - 1))
                    for p4 in range(4):
                        for jj in range(2):
                            j = half * 2 + jj
                            nc.scalar.copy(
                                qt_sb[p4][:, j * TQB:(j + 1) * TQB], qps[p4][jj])

                # ---- vT -> PE-transpose into v_sb ([T,64] layout + ones)
                for g in range(NG_LOC):
                    for t in range(NKT):
                        pst = psA.tile([128, TQB], F32, tag=f"b{(g * NKT + t) % 8}", name="pstT")
                        nc.tensor.transpose(
                            pst[:, 0:64],
                            vt_sb[g * 64:(g + 1) * 64, t * 128:(t + 1) * 128],
                            ident[g * 64:(g + 1) * 64, 0:64])
                        nc.vector.tensor_copy(
                            v_sb[g][:, t * 128 + 64:t * 128 + 128], pst[:, 0:64])

            # ================= Phase B: attention + proj =================
            with tc.tile_pool(name="pp", bufs=6) as ppool, \
                 tc.tile_pool(name="attn", bufs=8) as apool, \
                 tc.tile_pool(name="sm", bufs=4) as small, \
                 tc.tile_pool(name="yo", bufs=4) as ypool, \
                 tc.tile_pool(name="psS", bufs=2, space="PSUM") as psS, \
                 tc.tile_pool(name="psO", bufs=2, space="PSUM") as psO, \
                 tc.tile_pool(name="psP", bufs=2, space="PSUM") as psP:

                # --- outproj micro-op generator for one tq block ---
                # Yields thunks; one or two are drained after each attention
                # task so the PE fills ACT-pacing slack instead of starving
                # ACT with a monolithic projection burst at block end.
                def outproj_ops(j, at_prev):
                    for tt in range(4):
                        tau = j * 4 + tt
                        ysb = ypool.tile([128, C], F32, tag="y", name="ysb")
                        for half in range(2):
                            yp = psP.tile([128, TQB], F32, tag="yp", name="yp")
                            for p4 in range(4):
                                def mm(yp=yp, tt=tt, half=half, p4=p4):
                                    nc.tensor.matmul(
                                        yp,
                                        at_prev[p4][:, tt * 128:(tt + 1) * 128],
                                        wp_sb[p4][:, half * TQB:(half + 1) * TQB],
                                        start=(p4 == 0), stop=(p4 == 3))
                                yield mm
                            def cp(ysb=ysb, yp=yp, half=half):
                                nc.vector.tensor_copy(
                                    ysb[:, half * TQB:(half + 1) * TQB], yp)
                            yield cp
                        def dma(ysb=ysb, tau=tau):
                            nc.sync.dma_start(
                                out=y[tau * 128:(tau + 1) * 128, :], in_=ysb)
                        yield dma

                # --- software-pipelined attention ---
                at_prev = None
                pending = iter(())
                for j in range(NTQB):
                    tq0 = j * TQB
                    ntk = 4 * (j + 1)
                    npr = ntk // 2
                    at_j = [apool.tile([128, TQB], BF16, tag=f"at{p4}", name=f"at{p4}")
                            for p4 in range(4)]
                    tasks = [(h, pr) for h in range(NH_LOC) for pr in range(npr)]
                    if at_prev is not None:
                        pending = outproj_ops(j - 1, at_prev)
                    # ops drained per task: spread the previous block's 44
                    # outproj micro-ops evenly across this block's tasks
                    drain = (44 + len(tasks) - 1) // len(tasks)

                    def emit_scores(task):
                        h, pr = task
                        g = h // 4
                        p4, r = h // 2, h % 2
                        qT_h = qt_sb[p4][r * 64:(r + 1) * 64, :]
                        kT_g = kdup[g][r * 64:(r + 1) * 64, :]
                        psc = psS.tile([128, 2 * TQB], F32, tag="ps", name="psc")
                        pt = ppool.tile([128, 2 * TQB], BF16, tag="pt", name="ptp")
                        mem = []  # (t, off, base, width) packed layout
                        base = 0
                        for m in range(2):
                            t = 2 * pr + m
                            c = t - 4 * j
                            off = max(0, c * 128)
                            w = TQB - off
                            mem.append((t, off, base, w))
                            nc.tensor.matmul(
                                psc[:, base:base + w],
                                kT_g[:, t * 128:(t + 1) * 128],
                                qT_h[:, tq0 + off:tq0 + TQB],
                                start=True, stop=True)
                            base += w
                        return (psc, pt, mem, base)

                    def emit_rest(task, st, po):
                        h, pr = task
                        g = h // 4
                        psc, pt, mem, width = st
                        nc.scalar.activation(
                            pt[:, 0:width], psc[:, 0:width],
                            mybir.ActivationFunctionType.Exp, scale=SCALE)
                        for t, off, base, w in mem:
                            if t - 4 * j >= 0:
                                nc.vector.tensor_mul(
                                    pt[:, base:base + 128],
                                    pt[:, base:base + 128], mask)
                            nc.tensor.matmul(
                                po[:, off:TQB],
                                v_sb[g][:, t * 128:(t + 1) * 128],
                                pt[:, base:base + w],
                                start=(t == 0), stop=(t == ntk - 1))

                    st = emit_scores(tasks[0])
                    po = None
                    for i, task in enumerate(tasks):
                        h, pr = task
                        if pr == 0:
                            po = psO.tile([128, TQB], F32, tag="po", name="po")
                        st_next = emit_scores(tasks[i + 1]) if i + 1 < len(tasks) else None
                        emit_rest(task, st, po)
                        st = st_next
                        if pr == npr - 1:
                            # last pair of head h: normalize
                            p4, r = h // 2, h % 2
                            rcp = small.tile([128, TQB], F32, tag="recip", name="rcp")
                            nc.vector.reciprocal_approx_fast(
                                rcp[0:64, :], po[0:64, :])
                            nc.vector.tensor_mul(
                                at_j[p4][r * 64:(r + 1) * 64, :],
                                po[64:128, :], rcp[0:64, :])
                        for _ in range(drain):
                            op = next(pending, None)
                            if op is None:
                                break
                            op()
                    for op in pending:
                        op()
                    at_prev = at_j
                # tail: last block's output projection
                for op in outproj_ops(NTQB - 1, at_prev):
                    op()

    nc.compile()
    return nc


_NC_CACHE = None


def _get_nc():
    global _NC_CACHE
    if _NC_CACHE is None:
        _NC_CACHE = _build_program()
    return _NC_CACHE


def _make_in_maps(x, Wq, Wk, Wv, Wp):
    in_maps = []
    for core in range(8):
        b, tp = core // 2, core % 2
        hs = slice(tp * NH_LOC, (tp + 1) * NH_LOC)
        gs = slice(tp * NG_LOC, (tp + 1) * NG_LOC)
        in_maps.append({
            "xT": np.ascontiguousarray(x[b].T.astype(BFNP)),
            "wqT": np.ascontiguousarray(
                Wq[hs].transpose(2, 0, 1).reshape(C, S).astype(BFNP)),
            "wkT": np.ascontiguousarray(
                Wk[gs].transpose(2, 0, 1).reshape(C, NG_LOC * HD).astype(BFNP)),
            "wvT": np.ascontiguousarray(
                Wv[gs].transpose(2, 0, 1).reshape(C, NG_LOC * HD).astype(BFNP)),
            "wpT": np.ascontiguousarray(
                Wp[:, tp * S:(tp + 1) * S].T.astype(BFNP)),
        })
    return in_maps


def kernel(x, Wq, Wk, Wv, Wp, bp, _trace=False):
    x = np.asarray(x, dtype=np.float32)
    nc = _get_nc()
    in_maps = _make_in_maps(
        x, np.asarray(Wq, np.float32), np.asarray(Wk, np.float32),
        np.asarray(Wv, np.float32), np.asarray(Wp, np.float32))
    res = run_bass_kernel_spmd(nc, in_maps, list(range(8)), trace=_trace)
    out = np.empty((B, T, C), dtype=np.float32)
    bp32 = np.asarray(bp, np.float32)
    for b in range(B):
        out[b] = res.results[2 * b]["y"] + res.results[2 * b + 1]["y"] + bp32
    if _trace:
        return out, res
    return out


# revision 8
# speedup vs baseline: 1.4166x; 1.0363x over previous
"""GroupQueryAttention TRN2 Bass kernel (v2: all-bf16, ct-outer phase A,
paired activations).

Problem: B=4, T=2048, C=1024, H=16 heads, G=4 groups, head_dim=64, causal.
Sharding: 8 cores = 4 batches (DP) x 2 tensor-parallel halves (8 heads /
2 groups each). Host pre-transposes x and weight slices to bf16; each core
computes a partial output projection over its 512 attention channels; host
sums the two TP partials per batch and adds the bias.

v2 changes vs baseline (323-378us):
  - Everything bf16 on device (was fp32r projections): fp32-mode HIGH
    matmuls draw more power and trip the HAM throttle; bf16 streams at
    ~0.5ns/row. Host ships x/weights already in bf16 (halves input DMA).
  - Phase A is ct-outer: KV pass then two Q passes, each accumulating
    8 PSUM banks across the 8 contraction tiles, so the PE starts on the
    first x tile instead of waiting for the full 8MB x DMA.
  - Scores PSUM tiles are [128,1024] pairs (2 banks); one Exp ACTIVATE
    per pair halves the Scalar engine's 352-cycle/instruction overhead
    (ACT is the attention-phase co-bottleneck at ~1 elem/cycle/lane
    @1.2GHz). Clipped diagonal members leave stale PSUM in the dead
    columns; exp of garbage is finite-or-inf and never read (PV clips).
"""

import sys
import numpy as np
import ml_dtypes

for _p in ("/opt/trn_rl_repo", "/opt/trn_rl_repo/concourse"):
    if _p not in sys.path:
        sys.path.insert(0, _p)

import concourse.bass as bass  # noqa: E402
import concourse.mybir as mybir  # noqa: E402
from concourse import bacc  # noqa: E402
from concourse.tile import TileContext  # noqa: E402
from concourse.bass_utils import run_bass_kernel_spmd  # noqa: E402
from concourse.masks import make_identity, make_upper_triangular  # noqa: E402

F32 = mybir.dt.float32
BF16 = mybir.dt.bfloat16
BFNP = ml_dtypes.bfloat16

B, T, C = 4, 2048, 1024
NH, NG, HD = 16, 4, 64
NH_LOC, NG_LOC = 8, 2          # per-core heads / groups
S = NH_LOC * HD                # 512 local attention channels
TQB = 512                      # tq block
NTQB = T // TQB                # 4
NKT = T // 128                 # 16 tk tiles
NCT = C // 128                 # 8 contraction tiles
SCALE = float(HD) ** -0.5


def _build_program():
    nc = bacc.Bacc("TRN2", target_bir_lowering=False, debug=False, num_devices=8)

    xT = nc.dram_tensor("xT", [C, T], BF16, kind="ExternalInput")
    wqT = nc.dram_tensor("wqT", [C, S], BF16, kind="ExternalInput")
    wkT = nc.dram_tensor("wkT", [C, NG_LOC * HD], BF16, kind="ExternalInput")
    wvT = nc.dram_tensor("wvT", [C, NG_LOC * HD], BF16, kind="ExternalInput")
    wpT = nc.dram_tensor("wpT", [S, C], BF16, kind="ExternalInput")
    y = nc.dram_tensor("y", [T, C], F32, kind="ExternalOutput")

    with TileContext(nc) as tc:
        with tc.tile_pool(name="const", bufs=1) as const_pool, \
             tc.tile_pool(name="persist", bufs=1) as persist:

            ident = const_pool.tile([128, 64], F32)
            make_identity(nc, ident[0:64, 0:64])
            make_identity(nc, ident[64:128, 0:64], nomemset=False)
            mask32 = const_pool.tile([128, 128], F32)
            make_upper_triangular(nc, mask32, val=1.0, diag=True)
            mask = const_pool.tile([128, 128], BF16)
            nc.vector.tensor_copy(mask, mask32)
            ones64 = const_pool.tile([128, 64], F32)
            nc.vector.memset(ones64, 1.0)

            # ---- persistent SBUF tensors ----
            qt_sb = [persist.tile([128, T], BF16, tag=f"qt{i}", name=f"qt{i}")
                     for i in range(4)]
            kdup = [persist.tile([128, T], BF16, tag=f"kd{g}", name=f"kd{g}")
                    for g in range(NG_LOC)]
            # v (transposed back): per group 16 tiles [128, 128]; first 64
            # lhsT cols are ones so PV emits 64 replicated denominator rows
            v_sb = [persist.tile([128, NKT * 128], BF16, tag=f"v{g}", name=f"v{g}")
                    for g in range(NG_LOC)]
            wp_sb = [persist.tile([128, C], BF16, tag=f"wp{i}", name=f"wp{i}")
                     for i in range(4)]
            for g in range(NG_LOC):
                for t in range(NKT):
                    nc.vector.tensor_copy(
                        v_sb[g][:, t * 128:t * 128 + 64], ones64)

            # ================= Phase A: KV projections =================
            # xw stays open through phase B: the q projections and v
            # transposes are drained into the attention stream and still
            # need x / wq / vt tiles.
            with tc.tile_pool(name="xw", bufs=1) as xw:
                xts = [xw.tile([128, T], BF16, tag=f"x{ct}", name=f"x{ct}")
                       for ct in range(NCT)]
                wq_sb = [xw.tile([128, S], BF16, tag=f"wq{ct}", name=f"wq{ct}")
                         for ct in range(NCT)]
                wk_sb = [xw.tile([128, NG_LOC * HD], BF16, tag=f"wk{ct}", name=f"wk{ct}")
                        for ct in range(NCT)]
                wv_sb = [xw.tile([128, NG_LOC * HD], BF16, tag=f"wv{ct}", name=f"wv{ct}")
                        for ct in range(NCT)]
                vt_sb = xw.tile([128, T], F32, tag="vt", name="vt_sb")
                # ct-interleaved DMA issue so tile ct is complete before
                # the ct-th accumulation step of the kv pass
                for ct in range(NCT):
                    rows = slice(ct * 128, (ct + 1) * 128)
                    nc.sync.dma_start(out=xts[ct], in_=xT[rows, :])
                    nc.sync.dma_start(out=wk_sb[ct], in_=wkT[rows, :])
                    nc.sync.dma_start(out=wv_sb[ct], in_=wvT[rows, :])
                    nc.sync.dma_start(out=wq_sb[ct], in_=wqT[rows, :])
                for i in range(4):
                    nc.sync.dma_start(out=wp_sb[i], in_=wpT[i * 128:(i + 1) * 128, :])

                # ---- KV pass: 8 psum banks (k j0..3, v j0..3), ct-outer
                with tc.tile_pool(name="psA", bufs=1, space="PSUM") as psA:
                    kps = [psA.tile([128, TQB], F32, tag=f"b{j}", name=f"pk{j}")
                           for j in range(NTQB)]
                    vps = [psA.tile([128, TQB], F32, tag=f"b{4 + j}", name=f"pv{j}")
                           for j in range(NTQB)]
                    for ct in range(NCT):
                        for j in range(NTQB):
                            cols = slice(j * TQB, (j + 1) * TQB)
                            nc.tensor.matmul(
                                kps[j], wk_sb[ct], xts[ct][:, cols],
                                start=(ct == 0), stop=(ct == NCT - 1))
                            nc.tensor.matmul(
                                vps[j], wv_sb[ct], xts[ct][:, cols],
                                start=(ct == 0), stop=(ct == NCT - 1))
                    # k: duplicate each group onto both partition halves
                    for j in range(NTQB):
                        cols = slice(j * TQB, (j + 1) * TQB)
                        nc.scalar.copy(kdup[0][0:64, cols], kps[j][0:64, :])
                        nc.scalar.copy(kdup[1][64:128, cols], kps[j][64:128, :])
                    for j in range(NTQB):
                        cols = slice(j * TQB, (j + 1) * TQB)
                        nc.vector.tensor_copy(vt_sb[:, cols], vps[j])
                    nc.sync.dma_start(out=kdup[0][64:128, :], in_=kdup[0][0:64, :])
                    nc.sync.dma_start(out=kdup[1][0:64, :], in_=kdup[1][64:128, :])

                # ============ Phase B: attention + drained projections ======
                with tc.tile_pool(name="pp", bufs=6) as ppool, \
                     tc.tile_pool(name="attn", bufs=8) as apool, \
                     tc.tile_pool(name="sm", bufs=4) as small, \
                     tc.tile_pool(name="yo", bufs=4) as ypool, \
                     tc.tile_pool(name="psS", bufs=2, space="PSUM") as psS, \
                     tc.tile_pool(name="psO", bufs=2, space="PSUM") as psO:

                    qctr = [0]

                    def q_group_ops(psQ, p4, j):
                        """Accumulate q head-pair p4 for tq block j: 8 matmuls
                        + one DVE copy into qt_sb (Scalar stays free for exp)."""
                        ps = psQ.tile([128, TQB], F32, tag=f"q{qctr[0] % 2}",
                                      name="psq")
                        qctr[0] += 1
                        for ct in range(NCT):
                            def mm(ps=ps, ct=ct, p4=p4, j=j):
                                nc.tensor.matmul(
                                    ps,
                                    wq_sb[ct][:, p4 * 128:(p4 + 1) * 128],
                                    xts[ct][:, j * TQB:(j + 1) * TQB],
                                    start=(ct == 0), stop=(ct == NCT - 1))
                            yield mm
                        def cp(ps=ps, p4=p4, j=j):
                            nc.vector.tensor_copy(
                                qt_sb[p4][:, j * TQB:(j + 1) * TQB], ps)
                        yield cp

                    def transpose_ops(psQ, g, t):
                        ps = psQ.tile([128, TQB], F32, tag=f"q{qctr[0] % 2}",
                                      name="psq")
                        qctr[0] += 1
                        def tr(ps=ps, g=g, t=t):
                            nc.tensor.transpose(
                                ps[:, 0:64],
                                vt_sb[g * 64:(g + 1) * 64, t * 128:(t + 1) * 128],
                                ident[g * 64:(g + 1) * 64, 0:64])
                        yield tr
                        def cp(ps=ps, g=g, t=t):
                            nc.vector.tensor_copy(
                                v_sb[g][:, t * 128 + 64:t * 128 + 128],
                                ps[:, 0:64])
                        yield cp

                    def outproj_ops(psP, j, at_prev):
                        for tt in range(4):
                            tau = j * 4 + tt
                            ysb = ypool.tile([128, C], F32, tag="y", name="ysb")
                            for half in range(2):
                                yp = psP.tile([128, TQB], F32, tag="yp", name="yp")
                                for p4 in range(4):
                                    def mm(yp=yp, tt=tt, half=half, p4=p4):
                                        nc.tensor.matmul(
                                            yp,
                                            at_prev[p4][:, tt * 128:(tt + 1) * 128],
                                            wp_sb[p4][:, half * TQB:(half + 1) * TQB],
                                            start=(p4 == 0), stop=(p4 == 3))
                                    yield mm
                                def cp(ysb=ysb, yp=yp, half=half):
                                    nc.vector.tensor_copy(
                                        ysb[:, half * TQB:(half + 1) * TQB], yp)
                                yield cp
                            def dma(ysb=ysb, tau=tau):
                                nc.sync.dma_start(
                                    out=y[tau * 128:(tau + 1) * 128, :], in_=ysb)
                            yield dma

                    at_blocks = {}

                    def emit_block(j, pending, drain):
                        tq0 = j * TQB
                        ntk = 4 * (j + 1)
                        npr = ntk // 2
                        at_j = [apool.tile([128, TQB], BF16, tag=f"at{p4}",
                                           name=f"at{p4}") for p4 in range(4)]
                        at_blocks[j] = at_j
                        tasks = [(h, pr) for h in range(NH_LOC)
                                 for pr in range(npr)]

                        def emit_scores(task):
                            h, pr = task
                            g = h // 4
                            p4, r = h // 2, h % 2
                            qT_h = qt_sb[p4][r * 64:(r + 1) * 64, :]
                            kT_g = kdup[g][r * 64:(r + 1) * 64, :]
                            psc = psS.tile([128, 2 * TQB], F32, tag="ps", name="psc")
                            pt = ppool.tile([128, 2 * TQB], BF16, tag="pt", name="ptp")
                            mem = []  # (t, off, base, width) packed layout
                            base = 0
                            for m in range(2):
                                t = 2 * pr + m
                                c = t - 4 * j
                                off = max(0, c * 128)
                                w = TQB - off
                                mem.append((t, off, base, w))
                                nc.tensor.matmul(
                                    psc[:, base:base + w],
                                    kT_g[:, t * 128:(t + 1) * 128],
                                    qT_h[:, tq0 + off:tq0 + TQB],
                                    start=True, stop=True)
                                base += w
                            return (psc, pt, mem, base)

                        def emit_rest(task, st, po):
                            h, pr = task
                            g = h // 4
                            psc, pt, mem, width = st
                            nc.scalar.activation(
                                pt[:, 0:width], psc[:, 0:width],
                                mybir.ActivationFunctionType.Exp, scale=SCALE)
                            for t, off, base, w in mem:
                                if t - 4 * j >= 0:
                                    nc.vector.tensor_mul(
                                        pt[:, base:base + 128],
                                        pt[:, base:base + 128], mask)
                                nc.tensor.matmul(
                                    po[:, off:TQB],
                                    v_sb[g][:, t * 128:(t + 1) * 128],
                                    pt[:, base:base + w],
                                    start=(t == 0), stop=(t == ntk - 1))

                        st = emit_scores(tasks[0])
                        po = None
                        for i, task in enumerate(tasks):
                            h, pr = task
                            if pr == 0:
                                po = psO.tile([128, TQB], F32, tag="po", name="po")
                            st_next = (emit_scores(tasks[i + 1])
                                       if i + 1 < len(tasks) else None)
                            emit_rest(task, st, po)
                            st = st_next
                            if pr == npr - 1:
                                p4, r = h // 2, h % 2
                                rcp = small.tile([128, TQB], F32, tag="recip",
                                                 name="rcp")
                                nc.vector.reciprocal_approx_fast(
                                    rcp[0:64, :], po[0:64, :])
                                nc.vector.tensor_mul(
                                    at_j[p4][r * 64:(r + 1) * 64, :],
                                    po[64:128, :], rcp[0:64, :])
                            for _ in range(drain):
                                op = next(pending, None)
                                if op is None:
                                    break
                                op()
                        for op in pending:
                            op()

                    from itertools import chain as _chain

                    with tc.tile_pool(name="psQ", bufs=1, space="PSUM") as psQ:
                        # serial prologue: v transposes t0..3 + q for block 0
                        for g in range(NG_LOC):
                            for t in range(4):
                                for op in transpose_ops(psQ, g, t):
                                    op()
                        for p4 in range(4):
                            for op in q_group_ops(psQ, p4, 0):
                                op()
                        # block 0, draining q(j=1) head-pair 0 first (needed at
                        # the very start of block 1), then transposes t4..7,
                        # then the rest of q(j=1)
                        d0 = _chain(
                            q_group_ops(psQ, 0, 1),
                            *[transpose_ops(psQ, g, t)
                              for g in range(NG_LOC) for t in range(4, 8)],
                            *[q_group_ops(psQ, p4, 1) for p4 in range(1, 4)])
                        emit_block(0, d0, 4)
                        # block 1, draining q(j=2), transposes t8..15, q(j=3)
                        d1 = _chain(
                            q_group_ops(psQ, 0, 2),
                            *[transpose_ops(psQ, g, t)
                              for g in range(NG_LOC) for t in range(8, NKT)],
                            *[q_group_ops(psQ, p4, 2) for p4 in range(1, 4)],
                            *[q_group_ops(psQ, p4, 3) for p4 in range(4)])
                        emit_block(1, d1, 4)

                    with tc.tile_pool(name="psP", bufs=2, space="PSUM") as psP:
                        emit_block(2, _chain(outproj_ops(psP, 0, at_blocks[0]),
                                             outproj_ops(psP, 1, at_blocks[1])), 2)
                        emit_block(3, outproj_ops(psP, 2, at_blocks[2]), 1)
                        for op in outproj_ops(psP, 3, at_blocks[3]):
                            op()

    nc.compile()
    return nc


_NC_CACHE = None


def _get_nc():
    global _NC_CACHE
    if _NC_CACHE is None:
        _NC_CACHE = _build_program()
    return _NC_CACHE


def _make_in_maps(x, Wq, Wk, Wv, Wp):
    in_maps = []
    for core in range(8):
        b, tp = core // 2, core % 2
        hs = slice(tp * NH_LOC, (tp + 1) * NH_LOC)
        gs = slice(tp * NG_LOC, (tp + 1) * NG_LOC)
        in_maps.append({
            "xT": np.ascontiguousarray(x[b].T.astype(BFNP)),
            "wqT": np.ascontiguousarray(
                Wq[hs].transpose(2, 0, 1).reshape(C, S).astype(BFNP)),
            "wkT": np.ascontiguousarray(
                Wk[gs].transpose(2, 0, 1).reshape(C, NG_LOC * HD).astype(BFNP)),
            "wvT": np.ascontiguousarray(
                Wv[gs].transpose(2, 0, 1).reshape(C, NG_LOC * HD).astype(BFNP)),
            "wpT": np.ascontiguousarray(
                Wp[:, tp * S:(tp + 1) * S].T.astype(BFNP)),
        })
    return in_maps


def kernel(x, Wq, Wk, Wv, Wp, bp, _trace=False):
    x = np.asarray(x, dtype=np.float32)
    nc = _get_nc()
    in_maps = _make_in_maps(
        x, np.asarray(Wq, np.float32), np.asarray(Wk, np.float32),
        np.asarray(Wv, np.float32), np.asarray(Wp, np.float32))
    res = run_bass_kernel_spmd(nc, in_maps, list(range(8)), trace=_trace)
    out = np.empty((B, T, C), dtype=np.float32)
    bp32 = np.asarray(bp, np.float32)
    for b in range(B):
        out[b] = res.results[2 * b]["y"] + res.results[2 * b + 1]["y"] + bp32
    if _trace:
        return out, res
    return out
